# revision 17
# baseline (speedup 1.0000x reference)
"""Trainium2 Bass kernel for nn_Aggregation (sparse local attention aggregation).

out[n, g*64+cw, y, x] = sum_{i,j in 3x3} input[n, g*64+cw, y+i-1, x+j-1]
                        * weight[n, cw, i*3+j, y*64+x]

Sharding: data-parallel over batch n: 8 cores x 2 batches each.

Per-core layouts (host pre-swizzled, fp16 on the wire so HBM traffic is
halved; every DMA is a 2-dim [128 partitions x contiguous] transfer):
  x_t : [128=(b,cw), H, (g, 66)]  column-padded: [0, x0..x63, 0] per group,
        so the DMA band load lands directly as the zero-padded "even"
        shift plane (no on-chip scatter).
  w_t : [128=(b,cw), H, (ij, x)]
  o_t : [128=(b,cw), H, (g, x)]   fp16, host upcasts to f32.

Engine split per band (8 output rows), from trace analysis: the PE
identity-matmul accumulate costs ~0.77 ns/col (512-col matmul cap plus
~173 ns serial SBUF-access latency per matmul), DVE tensor ops run 2x
mode at ~0.54 ns/elem, Pool (gpsimd) multiplies at ~2 ns/elem. Balance:
  DMA : x/w band loads, out store (fp16).
  ACT : xe->xo shifted copy (odd-parity plane for aligned dj=0 reads),
        PSUM->SBUF evacuation with f32->fp16 cast.
  DVE : 7 of 9 shifted products + 2 tree-adds merging 3 planes into one
        partial sum.
  Pool: 2 of 9 products + edge-band halo memsets.
  PE  : 7 identity-matmul accumulate passes (6 raw planes + the DVE
        partial) into PSUM, fp16 moving data.
"""

import numpy as np

N, C, H, W = 16, 512, 64, 64
CW, G, K = 64, 8, 3
NCORE = 8
NB = N // NCORE          # batches per core
L = H * W

R = 8                    # band rows (output rows per band)
RP = R + 2               # plane rows incl. halo
NBANDS = H // R
WP = W + 2               # 66
GWP = G * WP             # one padded row-block (all groups)
WROW = K * K * W         # 576

POOL_IJ = (4,)           # center plane on Pool with fully-flat packed APs
                         # (Pool is 4-15x slower on strided/broadcast APs —
                         # software addressing on Q7 — but flat may be ok)
PROD_ORDER = (0, 3, 6, 2, 5, 8, 4, 1, 7)   # xe planes first, then flat-x

_cache = {}


def _build():
    import concourse.mybir as mybir
    from concourse import bacc
    from concourse.tile import TileContext
    from concourse.masks import make_identity

    f16 = mybir.dt.float16
    f32 = mybir.dt.float32

    nc = bacc.Bacc()
    x_t = nc.dram_tensor("x_t", [128, H, GWP], f16, kind="ExternalInput")
    w_t = nc.dram_tensor("w_t", [128, H, WROW], f16, kind="ExternalInput")
    o_t = nc.dram_tensor("o_t", [128, H, G * W], f16, kind="ExternalOutput")

    PL = RP * GWP            # padded plane length per partition

    with TileContext(nc) as tc:
        with (
            tc.tile_pool(name="const", bufs=1) as const_pool,
            tc.tile_pool(name="xe", bufs=3) as xe_pool,
            tc.tile_pool(name="wt", bufs=3) as wt_pool,
            tc.tile_pool(name="pr", bufs=2) as pr_pool,
            tc.tile_pool(name="os", bufs=2) as os_pool,
            tc.tile_pool(name="ps", bufs=1, space="PSUM") as ps_pool,
        ):
            # Two identity copies: alternating the stationary operand lets
            # each LDWEIGHTS target the background weight buffer and overlap
            # the in-flight matmul (same-tensor LDW serializes instead).
            ident = const_pool.tile([128, 128], f16)
            make_identity(nc, ident)
            ident2 = const_pool.tile([128, 128], f16)
            make_identity(nc, ident2)
            idents = [ident, ident2]

            for band in range(NBANDS):
                y0 = band * R
                row_lo = max(y0 - 1, 0)             # first loaded image row
                row_hi = min(y0 + R + 1, H)         # one past last loaded row
                RL = row_hi - row_lo                # rows loaded
                prow0 = 0 if y0 > 0 else 1          # plane row of first loaded row

                # ---- load x band straight into the padded even plane
                # (+66 slack: shifted APs over-run the last plane row)
                xe = xe_pool.tile([128, PL + 66], f16, tag="xe")
                nc.sync.dma_start(
                    out=xe[:, prow0 * GWP : (prow0 + RL) * GWP],
                    in_=x_t[:, row_lo:row_hi, :],
                )
                if band == 0:
                    nc.gpsimd.memset(xe[:, 0:GWP], 0.0)
                if band == NBANDS - 1:
                    nc.gpsimd.memset(xe[:, (RP - 1) * GWP : RP * GWP], 0.0)

                # ---- flat unpadded x copy: xf[r, g, c] = x (no pad cols).
                # Serves the dj=0 products: reads stay packed and 4-byte
                # aligned for DVE 2x, and are fully flat for the Pool plane.
                xf = xe_pool.tile([128, RP * G * W], f16, tag="xf")
                src = xe[:, :PL].rearrange("p (r g c) -> p r g c", g=G, c=WP)[
                    :, :, :, 1 : W + 1
                ]
                dst = xf[:].rearrange("p (r g c) -> p r g c", g=G, c=W)
                nc.scalar.copy(out=dst, in_=src)

                # ---- load weight band
                wt = wt_pool.tile([128, R * WROW], f16, tag="wt")
                nc.sync.dma_start(out=wt[:], in_=w_t[:, y0 : y0 + R, :])
                wv = wt[:].rearrange("p (r s) -> p r s", s=WROW)

                # ---- 9 shifted products (8 DVE, 1 Pool with flat APs)
                planes = [None] * (K * K)
                dve_tag = 0
                for ij in PROD_ORDER:
                    di, dj = ij // K - 1, ij % K - 1
                    if dj == 0:
                        xsrc = (
                            xf[:, (1 + di) * G * W : (1 + di + R) * G * W]
                            .rearrange("p (r g c) -> p r g c", g=G, c=W)
                        )
                    else:
                        col0 = 1 + dj               # dj=-1 -> 0, dj=+1 -> 2
                        off = (1 + di) * GWP + col0
                        xsrc = (
                            xe[:, off : off + R * GWP]
                            .rearrange("p (r gc) -> p r gc", gc=GWP)
                            .rearrange("p r (g c) -> p r g c", c=WP)[:, :, :, 0:W]
                        )
                    wsrc = (
                        wv[:, :, ij * W : (ij + 1) * W]
                        .unsqueeze(2)
                        .broadcast_to([128, R, G, W])
                    )
                    if ij in POOL_IJ:
                        # ACT expands the g-broadcast weight to a flat plane
                        # so the Pool multiply sees only packed 1-dim APs
                        wf = os_pool.tile([128, R * G * W], f16, tag="wf")
                        nc.scalar.copy(
                            out=wf[:].rearrange("p (r g c) -> p r g c", g=G, c=W),
                            in_=wsrc,
                        )
                        pr = pr_pool.tile([128, R * G * W], f16, tag="prP")
                        nc.gpsimd.tensor_mul(
                            out=pr[:],
                            in0=xf[:, (1 + di) * G * W : (1 + di + R) * G * W],
                            in1=wf[:],
                        )
                    else:
                        pr = pr_pool.tile([128, R * G * W], f16, tag=f"pr{dve_tag % 4}")
                        dve_tag += 1
                        prv = pr.rearrange("p (r g c) -> p r g c", g=G, c=W)
                        nc.vector.tensor_mul(out=prv, in0=xsrc, in1=wsrc)
                    planes[ij] = pr

                # ---- PE accumulate: all 9 planes in production order,
                # 8 x 512-col matmuls per pass across two 4-bank PSUM tiles
                passes = [planes[ij] for ij in PROD_ORDER]
                os_ = os_pool.tile([128, R * G * W], f16, tag="os")
                # Full-band accumulation: each pass consumes one whole plane
                # (8 matmuls across two 4-bank PSUM tiles), so product tiles
                # are consumed promptly and rotating-tag reuse cannot cycle.
                ps_tiles = [
                    ps_pool.tile(
                        [128, 4 * 512], f32, tag=f"ps{t}", name=f"ps_{band}_{t}"
                    )
                    for t in range(2)
                ]
                mm = 0
                for pi, pr in enumerate(passes):
                    for c in range(8):
                        nc.tensor.matmul(
                            ps_tiles[c // 4][:, (c % 4) * 512 : (c % 4 + 1) * 512],
                            idents[mm % 2],
                            pr[:, c * 512 : (c + 1) * 512],
                            start=(pi == 0),
                            stop=(pi == len(passes) - 1),
                        )
                        mm += 1
                # evacuate PSUM (f32 -> fp16)
                for t in range(2):
                    nc.scalar.copy(
                        out=os_[:, t * 2048 : (t + 1) * 2048], in_=ps_tiles[t]
                    )
                nc.sync.dma_start(out=o_t[:, y0 : y0 + R, :], in_=os_[:])

    nc.finalize()
    return nc


def _get():
    if "nc" not in _cache:
        _cache["nc"] = _build()
    return _cache["nc"]


def _swizzle_core(inp, wgt):
    # inp [2, 512, 64, 64] -> [128, H, G*66] fp16, zero-padded columns;
    # p = b*64+cw, free = (y, g, 66)
    a = inp.reshape(NB, G, CW, H, W).transpose(0, 2, 3, 1, 4)  # b,cw,y,g,x
    xe = np.zeros((NB, CW, H, G, WP), dtype=np.float16)
    xe[..., 1 : W + 1] = a
    xe = xe.reshape(128, H, GWP)
    # wgt [2, 64, 9, 4096] -> [128, H, 9*W] fp16; free = (y, ij, x)
    b = wgt.reshape(NB, CW, K * K, H, W).transpose(0, 1, 3, 2, 4)
    wt = np.ascontiguousarray(b, dtype=np.float16).reshape(128, H, WROW)
    return xe, wt


def _unswizzle_core(o):
    # [128, H, G*W] fp16 -> [2, 512, 64, 64] f32
    a = o.reshape(NB, CW, H, G, W).astype(np.float32).transpose(0, 3, 1, 2, 4)
    return np.ascontiguousarray(a).reshape(NB, C, H, W)


def kernel(input: np.ndarray, weight: np.ndarray) -> np.ndarray:
    from concourse.bass_utils import run_bass_kernel_spmd

    input = np.ascontiguousarray(input, dtype=np.float32)
    weight = np.ascontiguousarray(weight, dtype=np.float32)
    nc = _get()
    in_maps = []
    for i in range(NCORE):
        a, b = _swizzle_core(
            input[i * NB : (i + 1) * NB], weight[i * NB : (i + 1) * NB]
        )
        in_maps.append({"x_t": a, "w_t": b})
    res = run_bass_kernel_spmd(nc, in_maps, core_ids=list(range(NCORE)))
    return np.concatenate(
        [_unswizzle_core(res.results[i]["o_t"]) for i in range(NCORE)], axis=0
    )


# revision 19
# speedup vs baseline: 1.2291x; 1.2291x over previous
"""Trainium2 Bass kernel for nn_Aggregation (sparse local attention aggregation).

out[n, g*64+cw, y, x] = sum_{i,j in 3x3} input[n, g*64+cw, y+i-1, x+j-1]
                        * weight[n, cw, i*3+j, y*64+x]

Sharding: data-parallel over batch n: 8 cores x 2 batches each.

Per-core layouts (host pre-swizzled, fp16 on the wire so HBM traffic is
halved; every DMA is a 2-dim [128 partitions x contiguous] transfer):
  x_t : [128=(b,cw), H, (g, 66)]  column-padded: [0, x0..x63, 0] per group,
        so the DMA band load lands directly as the zero-padded shift plane
        (no on-chip scatter).
  w_t : [128=(b,cw), H, (ij, x)]
  o_t : [128=(b,cw), H, (g, x)]   fp16, host upcasts to f32.

Engine split per chunk of output rows, from trace analysis:
  DVE : all 9 shifted products (tensor_tensor fp16 2x mode, ~0.56 ns/elem
        — the binding engine at ~164 us/core; Pool's Q7 software multiply
        measured ~9.6 us/plane even on flat APs, 4x slower than DVE, so it
        only does the tiny edge memsets).
  PE  : 9 identity-matmul accumulate passes into PSUM. Back-to-back
        matmuls pipeline at ~216 ns per 512 cols, so PE has slack.
  ACT : xe->xf unpadded re-layout (keeps dj=0 reads 4-byte aligned for
        DVE 2x mode), PSUM->SBUF evacuation with f32->fp16 cast.
  DMA : x/w chunk loads, out store.
The first/last chunks are 4 rows instead of 8 to shorten pipeline
fill/drain, which the trace showed cost ~28 us combined.
"""

import numpy as np

N, C, H, W = 16, 512, 64, 64
CW, G, K = 64, 8, 3
NCORE = 8
NB = N // NCORE          # batches per core

R = 8                    # max chunk rows
RP = R + 2               # max plane rows incl. halo
WP = W + 2               # 66
GWP = G * WP             # one padded row-block (all groups)
WROW = K * K * W         # 576

PROD_ORDER = (0, 3, 6, 2, 5, 8, 4, 1, 7)   # xe planes first, then flat-x
CHUNKS = [4] + [8] * 7 + [4]               # head/tail chunks small to cut
                                           # pipeline fill/drain time

_cache = {}


def _build():
    import concourse.mybir as mybir
    from concourse import bacc
    from concourse.tile import TileContext
    from concourse.masks import make_identity

    f16 = mybir.dt.float16
    f32 = mybir.dt.float32

    nc = bacc.Bacc()
    x_t = nc.dram_tensor("x_t", [128, H, GWP], f16, kind="ExternalInput")
    w_t = nc.dram_tensor("w_t", [128, H, WROW], f16, kind="ExternalInput")
    o_t = nc.dram_tensor("o_t", [128, H, G * W], f16, kind="ExternalOutput")

    PL = RP * GWP            # padded plane length per partition (max chunk)

    with TileContext(nc) as tc:
        with (
            tc.tile_pool(name="const", bufs=1) as const_pool,
            tc.tile_pool(name="xe", bufs=3) as xe_pool,
            tc.tile_pool(name="wt", bufs=3) as wt_pool,
            tc.tile_pool(name="pr", bufs=2) as pr_pool,
            tc.tile_pool(name="os", bufs=2) as os_pool,
            tc.tile_pool(name="ps", bufs=1, space="PSUM") as ps_pool,
        ):
            # Two identity copies: alternating the stationary operand lets
            # each LDWEIGHTS target the background weight buffer and overlap
            # the in-flight matmul (same-tensor LDW serializes instead).
            ident = const_pool.tile([128, 128], f16)
            make_identity(nc, ident)
            ident2 = const_pool.tile([128, 128], f16)
            make_identity(nc, ident2)
            idents = [ident, ident2]
            # Warm the ACT function table during the boot phase so the
            # one-time ACT_TABLE_LOAD (~1.3us) doesn't delay the first
            # xe->xf copy.
            warm = const_pool.tile([128, 1], f16)
            nc.scalar.copy(out=warm[:], in_=ident[:, 0:1])

            y0 = 0
            for ci, Rc in enumerate(CHUNKS):
                RPc = Rc + 2
                row_lo = max(y0 - 1, 0)             # first loaded image row
                row_hi = min(y0 + Rc + 1, H)        # one past last loaded row
                RL = row_hi - row_lo                # rows loaded
                prow0 = 0 if y0 > 0 else 1          # plane row of first loaded row

                # ---- load x chunk straight into the padded shift plane
                # (+66 slack: shifted APs over-run the last plane row)
                xe = xe_pool.tile([128, PL + 66], f16, tag="xe")
                nc.sync.dma_start(
                    out=xe[:, prow0 * GWP : (prow0 + RL) * GWP],
                    in_=x_t[:, row_lo:row_hi, :],
                )
                if y0 == 0:
                    nc.gpsimd.memset(xe[:, 0:GWP], 0.0)
                if y0 + Rc == H:
                    nc.gpsimd.memset(xe[:, (RPc - 1) * GWP : RPc * GWP], 0.0)

                # ---- flat unpadded x copy: xf[r, g, c] = x (no pad cols);
                # keeps the dj=0 product reads packed and 4-byte aligned
                # for DVE 2x mode.
                xf = xe_pool.tile([128, RP * G * W], f16, tag="xf")
                src = xe[:, : RPc * GWP].rearrange(
                    "p (r g c) -> p r g c", g=G, c=WP
                )[:, :, :, 1 : W + 1]
                dst = xf[:, : RPc * G * W].rearrange(
                    "p (r g c) -> p r g c", g=G, c=W
                )
                nc.scalar.copy(out=dst, in_=src)

                # ---- load weight chunk
                wt = wt_pool.tile([128, R * WROW], f16, tag="wt")
                nc.sync.dma_start(
                    out=wt[:, : Rc * WROW], in_=w_t[:, y0 : y0 + Rc, :]
                )
                wv = wt[:, : Rc * WROW].rearrange("p (r s) -> p r s", s=WROW)

                # ---- 9 shifted products, all on DVE
                planes = []
                for pi, ij in enumerate(PROD_ORDER):
                    di, dj = ij // K - 1, ij % K - 1
                    if dj == 0:
                        xsrc = xf[
                            :, (1 + di) * G * W : (1 + di + Rc) * G * W
                        ].rearrange("p (r g c) -> p r g c", g=G, c=W)
                    else:
                        col0 = 1 + dj               # dj=-1 -> 0, dj=+1 -> 2
                        off = (1 + di) * GWP + col0
                        xsrc = (
                            xe[:, off : off + Rc * GWP]
                            .rearrange("p (r gc) -> p r gc", gc=GWP)
                            .rearrange("p r (g c) -> p r g c", c=WP)[:, :, :, 0:W]
                        )
                    wsrc = (
                        wv[:, :, ij * W : (ij + 1) * W]
                        .unsqueeze(2)
                        .broadcast_to([128, Rc, G, W])
                    )
                    pr = pr_pool.tile([128, R * G * W], f16, tag=f"pr{pi % 4}")
                    prv = pr[:, : Rc * G * W].rearrange(
                        "p (r g c) -> p r g c", g=G, c=W
                    )
                    nc.vector.tensor_mul(out=prv, in0=xsrc, in1=wsrc)
                    planes.append(pr)

                # ---- PE accumulate: all 9 planes in production order,
                # Rc x 512-col matmuls per pass across 4-bank PSUM tiles
                os_ = os_pool.tile([128, R * G * W], f16, tag="os")
                nt = (Rc + 3) // 4                  # PSUM tiles needed
                ps_tiles = [
                    ps_pool.tile(
                        [128, 4 * 512], f32, tag=f"ps{(ci + t) % 2}",
                        name=f"ps_{ci}_{t}",
                    )
                    for t in range(nt)
                ]
                mm = 0
                for pi, pr in enumerate(planes):
                    for c in range(Rc):
                        nc.tensor.matmul(
                            ps_tiles[c // 4][:, (c % 4) * 512 : (c % 4 + 1) * 512],
                            idents[mm % 2],
                            pr[:, c * 512 : (c + 1) * 512],
                            start=(pi == 0),
                            stop=(pi == len(planes) - 1),
                        )
                        mm += 1
                # ---- evacuate PSUM (f32 -> fp16) and store
                for t in range(nt):
                    hi = min(4 * 512, (Rc - 4 * t) * 512)
                    nc.scalar.copy(
                        out=os_[:, t * 2048 : t * 2048 + hi],
                        in_=ps_tiles[t][:, :hi],
                    )
                nc.sync.dma_start(
                    out=o_t[:, y0 : y0 + Rc, :], in_=os_[:, : Rc * G * W]
                )
                y0 += Rc

    nc.finalize()
    return nc


def _get():
    if "nc" not in _cache:
        _cache["nc"] = _build()
    return _cache["nc"]


def _swizzle_core(inp, wgt):
    # inp [2, 512, 64, 64] -> [128, H, G*66] fp16, zero-padded columns;
    # p = b*64+cw, free = (y, g, 66)
    a = inp.reshape(NB, G, CW, H, W).transpose(0, 2, 3, 1, 4)  # b,cw,y,g,x
    xe = np.zeros((NB, CW, H, G, WP), dtype=np.float16)
    xe[..., 1 : W + 1] = a
    xe = xe.reshape(128, H, GWP)
    # wgt [2, 64, 9, 4096] -> [128, H, 9*W] fp16; free = (y, ij, x)
    b = wgt.reshape(NB, CW, K * K, H, W).transpose(0, 1, 3, 2, 4)
    wt = np.ascontiguousarray(b, dtype=np.float16).reshape(128, H, WROW)
    return xe, wt


def _unswizzle_core(o):
    # [128, H, G*W] fp16 -> [2, 512, 64, 64] f32
    a = o.reshape(NB, CW, H, G, W).astype(np.float32).transpose(0, 3, 1, 2, 4)
    return np.ascontiguousarray(a).reshape(NB, C, H, W)


def kernel(input: np.ndarray, weight: np.ndarray) -> np.ndarray:
    from concourse.bass_utils import run_bass_kernel_spmd

    input = np.ascontiguousarray(input, dtype=np.float32)
    weight = np.ascontiguousarray(weight, dtype=np.float32)
    nc = _get()
    in_maps = []
    for i in range(NCORE):
        a, b = _swizzle_core(
            input[i * NB : (i + 1) * NB], weight[i * NB : (i + 1) * NB]
        )
        in_maps.append({"x_t": a, "w_t": b})
    res = run_bass_kernel_spmd(nc, in_maps, core_ids=list(range(NCORE)))
    return np.concatenate(
        [_unswizzle_core(res.results[i]["o_t"]) for i in range(NCORE)], axis=0
    )


# revision 27
# speedup vs baseline: 1.2637x; 1.0281x over previous
"""Trainium2 Bass kernel for nn_Aggregation (sparse local attention aggregation).

out[n, g*64+cw, y, x] = sum_{i,j in 3x3} input[n, g*64+cw, y+i-1, x+j-1]
                        * weight[n, cw, i*3+j, y*64+x]

Sharding: data-parallel over batch n: 8 cores x 2 batches each.

Per-core layouts (host pre-swizzled, fp16 on the wire so HBM traffic is
halved; every DMA is a 2-dim [128 partitions x contiguous] transfer):
  x_t : [128=(b,cw), H, (g, 66)]  column-padded: [0, x0..x63, 0] per group,
        so the DMA band load lands directly as the zero-padded shift plane
        (no on-chip scatter).
  w_t : [128=(b,cw), H, (ij, x)]
  o_t : [128=(b,cw), H, (g, x)]   fp16, host upcasts to f32.

Engine split per chunk of output rows, from trace analysis:
  DVE : all 9 shifted products (tensor_tensor fp16 2x mode, ~0.56 ns/elem
        — the binding engine at ~164 us/core; Pool's Q7 software multiply
        measured ~9.6 us/plane even on flat APs, 4x slower than DVE, so it
        only does the tiny edge memsets).
  PE  : 9 identity-matmul accumulate passes into PSUM. Back-to-back
        matmuls pipeline at ~216 ns per 512 cols, so PE has slack.
  ACT : xe->xf unpadded re-layout (keeps dj=0 reads 4-byte aligned for
        DVE 2x mode), PSUM->SBUF evacuation with f32->fp16 cast.
  DMA : x/w chunk loads, out store.
The first/last chunks are 4 rows instead of 8 to shorten pipeline
fill/drain, which the trace showed cost ~28 us combined.
"""

import numpy as np

N, C, H, W = 16, 512, 64, 64
CW, G, K = 64, 8, 3
NCORE = 8
NB = N // NCORE          # batches per core

R = 8                    # max chunk rows
RP = R + 2               # max plane rows incl. halo
WP = W + 2               # 66
GWP = G * WP             # one padded row-block (all groups)
WROW = K * K * W         # 576

PROD_ORDER = (0, 3, 6, 2, 5, 8, 4, 1, 7)   # xe planes first, then flat-x
CHUNKS = [4] + [8] * 7 + [4]               # head/tail chunks small to cut
                                           # pipeline fill/drain time

_cache = {}


def _build():
    import concourse.mybir as mybir
    from concourse import bacc
    from concourse.tile import TileContext
    from concourse.masks import make_identity

    f16 = mybir.dt.float16
    f32 = mybir.dt.float32

    nc = bacc.Bacc()
    x_t = nc.dram_tensor("x_t", [128, H, GWP], f16, kind="ExternalInput")
    w_t = nc.dram_tensor("w_t", [128, H, WROW], f16, kind="ExternalInput")
    o_t = nc.dram_tensor("o_t", [128, H, G * W], f16, kind="ExternalOutput")

    PL = RP * GWP            # padded plane length per partition (max chunk)

    with TileContext(nc) as tc:
        with (
            tc.tile_pool(name="const", bufs=1) as const_pool,
            tc.tile_pool(name="xe", bufs=3) as xe_pool,
            tc.tile_pool(name="wt", bufs=3) as wt_pool,
            tc.tile_pool(name="pr", bufs=2) as pr_pool,
            tc.tile_pool(name="os", bufs=2) as os_pool,
            tc.tile_pool(name="ps", bufs=1, space="PSUM") as ps_pool,
        ):
            # Two identity copies: alternating the stationary operand lets
            # each LDWEIGHTS target the background weight buffer and overlap
            # the in-flight matmul (same-tensor LDW serializes instead).
            ident = const_pool.tile([128, 128], f16)
            make_identity(nc, ident)
            ident2 = const_pool.tile([128, 128], f16)
            make_identity(nc, ident2)
            idents = [ident, ident2]
            # Warm the ACT function table during the boot phase so the
            # one-time ACT_TABLE_LOAD (~1.3us) doesn't delay the first
            # xe->xf copy.
            warm = const_pool.tile([128, 1], f16)
            nc.scalar.copy(out=warm[:], in_=ident[:, 0:1])

            y0 = 0
            for ci, Rc in enumerate(CHUNKS):
                RPc = Rc + 2
                row_lo = max(y0 - 1, 0)             # first loaded image row
                row_hi = min(y0 + Rc + 1, H)        # one past last loaded row
                RL = row_hi - row_lo                # rows loaded
                prow0 = 0 if y0 > 0 else 1          # plane row of first loaded row

                # ---- load x chunk straight into the padded shift plane
                # (+66 slack: shifted APs over-run the last plane row)
                xe = xe_pool.tile([128, PL + 66], f16, tag="xe")
                nc.sync.dma_start(
                    out=xe[:, prow0 * GWP : (prow0 + RL) * GWP],
                    in_=x_t[:, row_lo:row_hi, :],
                )
                if y0 == 0:
                    nc.gpsimd.memset(xe[:, 0:GWP], 0.0)
                if y0 + Rc == H:
                    nc.gpsimd.memset(xe[:, (RPc - 1) * GWP : RPc * GWP], 0.0)

                # ---- flat unpadded x copy: xf[r, g, c] = x (no pad cols);
                # keeps the dj=0 product reads packed and 4-byte aligned
                # for DVE 2x mode.
                xf = xe_pool.tile([128, RP * G * W], f16, tag="xf")
                src = xe[:, : RPc * GWP].rearrange(
                    "p (r g c) -> p r g c", g=G, c=WP
                )[:, :, :, 1 : W + 1]
                dst = xf[:, : RPc * G * W].rearrange(
                    "p (r g c) -> p r g c", g=G, c=W
                )
                nc.scalar.copy(out=dst, in_=src)

                # ---- load weight chunk
                wt = wt_pool.tile([128, R * WROW], f16, tag="wt")
                nc.sync.dma_start(
                    out=wt[:, : Rc * WROW], in_=w_t[:, y0 : y0 + Rc, :]
                )
                wv = wt[:, : Rc * WROW].rearrange("p (r s) -> p r s", s=WROW)

                # ---- 9 shifted products, all on DVE
                planes = []
                for pi, ij in enumerate(PROD_ORDER):
                    di, dj = ij // K - 1, ij % K - 1
                    if dj == 0:
                        xsrc = xf[
                            :, (1 + di) * G * W : (1 + di + Rc) * G * W
                        ].rearrange("p (r g c) -> p r g c", g=G, c=W)
                    else:
                        col0 = 1 + dj               # dj=-1 -> 0, dj=+1 -> 2
                        off = (1 + di) * GWP + col0
                        xsrc = (
                            xe[:, off : off + Rc * GWP]
                            .rearrange("p (r gc) -> p r gc", gc=GWP)
                            .rearrange("p r (g c) -> p r g c", c=WP)[:, :, :, 0:W]
                        )
                    wsrc = (
                        wv[:, :, ij * W : (ij + 1) * W]
                        .unsqueeze(2)
                        .broadcast_to([128, Rc, G, W])
                    )
                    pr = pr_pool.tile([128, R * G * W], f16, tag=f"pr{pi % 4}")
                    prv = pr[:, : Rc * G * W].rearrange(
                        "p (r g c) -> p r g c", g=G, c=W
                    )
                    nc.vector.tensor_mul(out=prv, in0=xsrc, in1=wsrc)
                    planes.append(pr)

                # ---- PE accumulate: all 9 planes in production order,
                # Rc x 512-col matmuls per pass across 4-bank PSUM tiles
                os_ = os_pool.tile([128, R * G * W], f16, tag="os")
                nt = (Rc + 3) // 4                  # PSUM tiles needed
                ps_tiles = [
                    ps_pool.tile(
                        [128, 4 * 512], f32, tag=f"ps{(ci + t) % 2}",
                        name=f"ps_{ci}_{t}",
                    )
                    for t in range(nt)
                ]
                mm = 0
                for pi, pr in enumerate(planes):
                    for c in range(Rc):
                        nc.tensor.matmul(
                            ps_tiles[c // 4][:, (c % 4) * 512 : (c % 4 + 1) * 512],
                            idents[mm % 2],
                            pr[:, c * 512 : (c + 1) * 512],
                            start=(pi == 0),
                            stop=(pi == len(planes) - 1),
                        )
                        mm += 1
                # ---- evacuate PSUM (f32 -> fp16) and store
                for t in range(nt):
                    hi = min(4 * 512, (Rc - 4 * t) * 512)
                    nc.scalar.copy(
                        out=os_[:, t * 2048 : t * 2048 + hi],
                        in_=ps_tiles[t][:, :hi],
                    )
                nc.sync.dma_start(
                    out=o_t[:, y0 : y0 + Rc, :], in_=os_[:, : Rc * G * W]
                )
                y0 += Rc

    nc.finalize()
    return nc


def _get():
    if "nc" not in _cache:
        _cache["nc"] = _build()
    return _cache["nc"]


def _swizzle_core(inp, wgt):
    # inp [2, 512, 64, 64] -> [128, H, G*66] fp16, zero-padded columns;
    # p = b*64+cw, free = (y, g, 66)
    a = inp.reshape(NB, G, CW, H, W).transpose(0, 2, 3, 1, 4)  # b,cw,y,g,x
    xe = np.zeros((NB, CW, H, G, WP), dtype=np.float16)
    xe[..., 1 : W + 1] = a
    xe = xe.reshape(128, H, GWP)
    # wgt [2, 64, 9, 4096] -> [128, H, 9*W] fp16; free = (y, ij, x)
    b = wgt.reshape(NB, CW, K * K, H, W).transpose(0, 1, 3, 2, 4)
    wt = np.ascontiguousarray(b, dtype=np.float16).reshape(128, H, WROW)
    return xe, wt


def _unswizzle_core(o):
    # [128, H, G*W] fp16 -> [2, 512, 64, 64] f32
    a = o.reshape(NB, CW, H, G, W).astype(np.float32).transpose(0, 3, 1, 2, 4)
    return np.ascontiguousarray(a).reshape(NB, C, H, W)


def kernel(input: np.ndarray, weight: np.ndarray) -> np.ndarray:
    from concourse.bass_utils import run_bass_kernel_spmd

    input = np.ascontiguousarray(input, dtype=np.float32)
    weight = np.ascontiguousarray(weight, dtype=np.float32)
    nc = _get()
    in_maps = []
    for i in range(NCORE):
        a, b = _swizzle_core(
            input[i * NB : (i + 1) * NB], weight[i * NB : (i + 1) * NB]
        )
        in_maps.append({"x_t": a, "w_t": b})
    res = run_bass_kernel_spmd(nc, in_maps, core_ids=list(range(NCORE)))
    return np.concatenate(
        [_unswizzle_core(res.results[i]["o_t"]) for i in range(NCORE)], axis=0
    )
